# revision 31
# baseline (speedup 1.0000x reference)
"""Trainium2 Bass kernel for nn_CrossAttentionFusion.

Math: softmax over kv_len==1 is identically 1.0, so the attention output is
v broadcast over the N (patch) axis and the whole module reduces to

    out[b, n, :] = cnn[b] @ (Wkv[:, C:] @ Wp) + bp        (independent of n)

W_eff = Wkv[:, C:] @ Wp is a weight-only constant, folded on the host.

Strategy: data-parallel over batch B=64 across 8 NeuronCores (8 batches per
core), W_eff replicated. The 768 output columns are computed in two balanced
passes (384 + 384), each with its own contiguous W_eff slab so pass 0's
weights land first; pass-0 broadcast DMAs start while pass-1 weights are
still streaming in. Scratch warm-up matmuls lift the PE HAM throttle before
the real matmuls arrive. Per (pass, batch) a one-hot matmul replicates row[b]
across 128 SBUF partitions and stride-0-source broadcast DMAs on both HWDGE
rings write the (576, 384) output block.
"""

import sys

sys.path.insert(0, "/opt/trn_rl_repo")

import numpy as np

import concourse.bass as bass
import concourse.mybir as mybir
from concourse import bacc
from concourse.bass_utils import run_bass_kernel_spmd
from concourse.tile import TileContext

F32 = mybir.dt.float32

NCORES = 8
B, N, C, CNN = 64, 576, 768, 2048
BS = B // NCORES  # batches per core = 8
KC = CNN // 128  # 16 k-chunks
CWA, CWB = 256, 512  # columns per pass: small pass first so DMAs start early
HALVES = ((0, CWA), (CWA, C))


def _build_bass():
    nc = bacc.Bacc(None, target_bir_lowering=False, debug=False, num_devices=NCORES)

    x_cnnT = nc.declare_dram_parameter("cnnT", [128, KC * BS], F32, isOutput=False)
    x_weffA = nc.declare_dram_parameter("weffA", [128, KC * CWA], F32, isOutput=False)
    x_weffB = nc.declare_dram_parameter("weffB", [128, KC * CWB], F32, isOutput=False)
    x_bpb = nc.declare_dram_parameter("bpb", [BS, C], F32, isOutput=False)
    x_sel = nc.declare_dram_parameter("sel", [BS, BS * 128], F32, isOutput=False)
    y = nc.declare_dram_parameter("out", [BS, N, C], F32, isOutput=True)

    with TileContext(nc) as tc:
        with (
            tc.tile_pool(name="singles", bufs=1) as singles,
            tc.tile_pool(name="psum_r", bufs=1, space="PSUM") as psum_r,
            tc.tile_pool(name="psum_bc", bufs=5, space="PSUM") as psum_bc,
            tc.tile_pool(name="bc_sb", bufs=8) as bc_sb,
        ):
            # PE warm-up: junk matmuls on scratch data lift the HAM throttle
            # (~3.4 us busy window) before the real matmuls arrive.
            wu_sb = singles.tile([128, 512], F32, tag="wu_sb")
            nc.gpsimd.memset(wu_sb[:], 0.0)
            with tc.tile_pool(name="psum_w", bufs=1, space="PSUM") as psum_w:
                ps_w = psum_w.tile([BS, 512], F32, tag="ps_w")
                for _ in range(2):
                    nc.tensor.matmul(
                        ps_w[:], wu_sb[:, 0:BS], wu_sb[:, :], start=True, stop=True
                    )

            cnnT_t = singles.tile([128, KC * BS], F32, tag="cnnT")
            nc.sync.dma_start(out=cnnT_t[:], in_=x_cnnT[:, :])
            # pass-0 columns first in fine chunks (2 k-chunks each, 256 KB)
            # so the PE can start as soon as possible; pass-1 in 1 MB chunks.
            weffA_t = []
            for g in range(8):
                wt = singles.tile([128, 2 * CWA], F32, tag=f"weffA{g}", name=f"weffA{g}")
                nc.sync.dma_start(
                    out=wt[:], in_=x_weffA[:, g * 2 * CWA : (g + 1) * 2 * CWA]
                )
                weffA_t.append(wt)
            weffB_t = []
            for g in range(4):
                wt = singles.tile([128, 4 * CWB], F32, tag=f"weffB{g}", name=f"weffB{g}")
                nc.sync.dma_start(
                    out=wt[:], in_=x_weffB[:, g * 4 * CWB : (g + 1) * 4 * CWB]
                )
                weffB_t.append(wt)
            sel_t = singles.tile([BS, BS * 128], F32, tag="sel")
            nc.scalar.dma_start(out=sel_t[:], in_=x_sel[:, :])
            bpb_t = singles.tile([BS, C], F32, tag="bpb")
            nc.scalar.dma_start(out=bpb_t[:], in_=x_bpb[:, :])

            row_t = singles.tile([BS, C], F32, tag="row")
            ps_rows = [
                psum_r.tile([BS, CWA], F32, tag="ps_rowA", name="ps_rowA"),
                psum_r.tile([BS, CWB], F32, tag="ps_rowB", name="ps_rowB"),
            ]

            for half, (c0, c1) in enumerate(HALVES):
                cw = c1 - c0
                wtiles = weffA_t if half == 0 else weffB_t
                kc_per = 2 if half == 0 else 4
                ps_row = ps_rows[half]
                # Stage pass: row[:, c0:c1] = cnn_shard @ W_eff[:, c0:c1]
                for kc in range(KC):
                    wt = wtiles[kc // kc_per]
                    w0 = (kc % kc_per) * cw
                    nc.tensor.matmul(
                        ps_row[:],
                        cnnT_t[:, kc * BS : (kc + 1) * BS],
                        wt[:, w0 : w0 + cw],
                        start=(kc == 0),
                        stop=(kc == KC - 1),
                    )
                nc.vector.tensor_add(
                    row_t[:, c0:c1], ps_row[:], bpb_t[:, c0:c1]
                )

                for b in range(BS):
                    ps_bc = psum_bc.tile([128, cw], F32, name="ps_bc", tag="ps_bc")
                    nc.tensor.matmul(
                        ps_bc[:],
                        sel_t[:, b * 128 : (b + 1) * 128],
                        row_t[:, c0:c1],
                        start=True,
                        stop=True,
                    )
                    bc_t = bc_sb.tile([128, cw], F32, name="bc_t", tag="bc_t")
                    nc.vector.tensor_copy(bc_t[:], ps_bc[:])

                    # rows 0..511: n = 4*p + j, 128 partitions, stride-0 j.
                    src_a = bc_t[:, :].unsqueeze(1).broadcast_to((128, 4, cw))
                    dst_a = y[b, 0:512, c0:c1].rearrange("(p j) c -> p j c", j=4)
                    # rows 512..575 from 64 partitions (alternate halves).
                    h0 = 0 if b % 2 == 0 else 64
                    src_b = bc_t[h0 : h0 + 64, :]
                    dst_b = y[b, 512:N, c0:c1]
                    eng_a = nc.sync if b % 2 == 0 else nc.scalar
                    eng_b = nc.scalar if b % 2 == 0 else nc.sync
                    eng_a.dma_start(out=dst_a, in_=src_a)
                    eng_b.dma_start(out=dst_b, in_=src_b)

    nc.compile()
    return nc


_NC = None


def _get_nc():
    global _NC
    if _NC is None:
        _NC = _build_bass()
    return _NC


def _prepare_in_maps(image_patches, cnn_feature_vector, Wq, Wkv, Wp, bp):
    Weff = np.ascontiguousarray(Wkv[:, C:]) @ Wp  # (2048, 768) fp32
    weffA_arr = np.ascontiguousarray(
        Weff[:, 0:CWA].reshape(KC, 128, CWA).transpose(1, 0, 2).reshape(128, KC * CWA)
    )
    weffB_arr = np.ascontiguousarray(
        Weff[:, CWA:C].reshape(KC, 128, CWB).transpose(1, 0, 2).reshape(128, KC * CWB)
    )
    bpb = np.ascontiguousarray(np.broadcast_to(bp.astype(np.float32), (BS, C)))
    sel = np.zeros((BS, BS * 128), dtype=np.float32)
    for b in range(BS):
        sel[b, b * 128 : (b + 1) * 128] = 1.0

    in_maps = []
    for core in range(NCORES):
        shard = cnn_feature_vector[core * BS : (core + 1) * BS]  # (8, 2048)
        cnnT = np.ascontiguousarray(
            shard.T.reshape(KC, 128, BS).transpose(1, 0, 2).reshape(128, KC * BS)
        )
        in_maps.append(
            {
                "cnnT": cnnT,
                "weffA": weffA_arr,
                "weffB": weffB_arr,
                "bpb": bpb,
                "sel": sel,
            }
        )
    return in_maps


def kernel(**inputs) -> np.ndarray:
    inputs = {k: np.asarray(v) for k, v in inputs.items()}
    nc = _get_nc()
    in_maps = _prepare_in_maps(**inputs)
    res = run_bass_kernel_spmd(nc, in_maps, core_ids=list(range(NCORES)))
    return np.concatenate([res.results[i]["out"] for i in range(NCORES)], axis=0)


def kernel_traced(**inputs):
    """kernel() + HW profile; returns (output, BassKernelResults)."""
    inputs = {k: np.asarray(v) for k, v in inputs.items()}
    nc = _get_nc()
    in_maps = _prepare_in_maps(**inputs)
    res = run_bass_kernel_spmd(
        nc, in_maps, core_ids=list(range(NCORES)), trace=True
    )
    out = np.concatenate([res.results[i]["out"] for i in range(NCORES)], axis=0)
    return out, res
